# revision 9
# baseline (speedup 1.0000x reference)
"""Trainium2 Bass kernel for nn_KANStressPredictor.

Computes, per element-triple (s0, s1, s2) of `strain` [B, T, 3]:
    c00 = 2*s0+1, c11 = 2*s1+1, c01 = s2          (C = 2E + I, sym 2x2)
    t1, t2 = eigenvalues of C   (t = mean -/+ rad)
    out0, out1 = (sqrt(t_i) * det^(-1/6))^ki0
    out2       = ki1 * 0.5 * log(det)

Key algebraic reductions used here:
    mean = s0+s1+1,  rad^2 = (s0-s1)^2 + s2^2,  t_i = (s0+s1) -/+ rad + 1
    det  = t1*t2  =>  log(det) = log(t1) + log(t2)   (never materialized)
    out_i = exp(ki0/6 * (3*log(t_i) - L)),  L = log(t1)+log(t2)
    rad  = exp(0.5*ln(rad^2))  -- keeps ACT in one table set (ln/exp only)

Sharding: pure data-parallel over the batch dim across 8 cores. Each core's
shard is viewed as [128, F] (partition-contiguous), processed in free-dim
chunks with interleaved-triple strided access patterns.
"""

import sys

for _p in ("/opt/trn_rl_repo",):
    if _p not in sys.path:
        sys.path.insert(0, _p)

import numpy as np

import concourse.bacc as bacc
import concourse.bass as bass
import concourse.tile as tile
from concourse import mybir
from concourse.bass_utils import run_bass_kernel_spmd

N_CORES = 8
P = 128

_cache: dict = {}


def _build(ki0: float, ki1: float, F: int, chunk_triples: int, reps: int = 1):
    """Build + compile the Bass program for one core's [P, F] shard.

    reps > 1 repeats the whole pipeline (same input/output) for benchmarking:
    marginal time per rep = steady-state exec time with dispatch cancelled.
    """
    key = (ki0, ki1, F, chunk_triples, reps)
    if key in _cache:
        return _cache[key]

    f32 = mybir.dt.float32
    AF = mybir.ActivationFunctionType
    Add = mybir.AluOpType.add
    Sub = mybir.AluOpType.subtract
    Mult = mybir.AluOpType.mult

    CT = chunk_triples
    CE = CT * 3  # elems per chunk per partition
    assert F % CE == 0
    n_chunks = F // CE

    nc = bacc.Bacc("TRN2", target_bir_lowering=False, debug=False)
    in_ap = nc.dram_tensor("strain", [P, F], f32, kind="ExternalInput").ap()
    out_ap = nc.dram_tensor("out", [P, F], f32, kind="ExternalOutput").ap()

    with tile.TileContext(nc) as tc:
        with (
            tc.tile_pool(name="io", bufs=2) as iop,
            tc.tile_pool(name="pl", bufs=3) as pl,
        ):
            for ci in range(n_chunks * reps):
                ci = ci % n_chunks
                sl = bass.ts(ci, CE)
                I = iop.tile([P, CE], f32, name="in", tag="in")
                nc.sync.dma_start(I[:], in_ap[:, sl])
                I3 = I[:].rearrange("p (n k) -> p k n", k=3)
                a, b, c = I3[:, 0], I3[:, 1], I3[:, 2]

                O = iop.tile([P, CE], f32, name="out", tag="out")
                Opair = O[:].rearrange("p (n k) -> p n k", k=3)[:, :, 0:2]
                Oc = O[:].rearrange("p (n k) -> p k n", k=3)[:, 2]

                def T(tag, width=CT):
                    return pl.tile([P, width], f32, name=tag, tag=tag)[:]

                s = T("s")
                nc.vector.tensor_add(s, a, b)  # s0+s1
                u = T("u")
                nc.vector.tensor_sub(u, a, b)  # s0-s1
                q = T("q")
                nc.scalar.activation(q, c, AF.Square)  # s2^2
                nc.scalar.activation(u, u, AF.Square)  # (s0-s1)^2, in place
                r2 = T("r2")
                nc.vector.tensor_add(r2, u, q)  # rad^2
                nc.scalar.activation(r2, r2, AF.Ln)  # in place
                rad = T("rad")
                nc.scalar.activation(rad, r2, AF.Exp, scale=0.5)  # sqrt(rad^2)

                D = T("D", 2 * CT)  # (d1, d2) interleaved pairs
                Dp = D[:].rearrange("p (n k) -> p n k", k=2)
                nc.vector.scalar_tensor_tensor(
                    Dp[:, :, 0], rad, -1.0, s, Mult, Add
                )  # d1 = s - rad
                nc.vector.tensor_add(Dp[:, :, 1], s, rad)  # d2 = s + rad
                # l = ln(d + 1) for both eigenvalues in one pass
                nc.scalar.activation(D[:], D[:], AF.Ln, bias=1.0)
                l3 = D[:].rearrange("p (n k) -> p n k", k=2)
                l1, l2 = l3[:, :, 0], l3[:, :, 1]
                lswap = l3[:, :, ::-1]
                L = T("L")
                nc.vector.tensor_add(L, l1, l2)  # log(det)
                # w_i = l_i - 0.5*l_other; out_i = exp(ki0/3 * w_i)
                W = T("W", 2 * CT)
                Wp = W[:].rearrange("p (n k) -> p n k", k=2)
                nc.vector.scalar_tensor_tensor(Wp, lswap, -0.5, l3, Mult, Add)
                nc.scalar.activation(Opair, Wp, AF.Exp, scale=ki0 / 3.0)
                nc.scalar.mul(Oc, L, ki1 * 0.5)

                nc.sync.dma_start(out_ap[:, sl], O[:])

    nc.compile()
    _cache[key] = nc
    return nc


def _run(strain: np.ndarray, ki0: float, ki1: float, trace: bool = False,
         chunk_triples: int = 1024):
    B, T, C = strain.shape
    assert C == 3 and B % N_CORES == 0
    Bs = B // N_CORES
    elems = Bs * T * C
    assert elems % P == 0
    F = elems // P
    assert (F // 3) % chunk_triples == 0

    nc = _build(float(ki0), float(ki1), F, chunk_triples)

    flat = np.ascontiguousarray(strain, dtype=np.float32).reshape(N_CORES, P, F)
    in_maps = [{"strain": flat[i]} for i in range(N_CORES)]
    res = run_bass_kernel_spmd(nc, in_maps, list(range(N_CORES)), trace=trace)
    out = np.stack([np.asarray(res.results[i]["out"]) for i in range(N_CORES)])
    out = out.reshape(B, T, C).astype(np.float32, copy=False)
    return out, res


def kernel(strain: np.ndarray, ki0, ki1) -> np.ndarray:
    out, _ = _run(np.asarray(strain), float(np.asarray(ki0)), float(np.asarray(ki1)))
    return out
